# revision 12
# baseline (speedup 1.0000x reference)
"""MultiHeadAttention (cross-attention, B=32 N=512 L=1024 D=512 H=8) on 8 TRN2 cores.

Strategy: data parallelism (4 batches/core) + host-side sparsity compaction.

Host prep (inside kernel(), plain numpy):
  - per batch, gather the unmasked K/V positions (~50% of L=1024), pad to
    L_C=640 (5*128); padded slots get zero K/V rows and a -87 exp bias so they
    vanish from the softmax exactly like reference's -inf masking
  - rpb rows gathered the same way; x_q / x_kv / rpb pre-TRANSPOSED on host so
    the device needs no PE transposes at all
Device per-core dataflow (all matmuls float32r, 1 cycle/row on PE):
  Q^T/K^T (+rpb^T via DVE add) head-major; V natural with interleaved ones col
  scores S^T[l,n] per head-pair packed via tile_position (K=64 row groups),
  both heads' scores in one [128,1024] PSUM tile -> single exp per (pair,chunk)
  exp on ACT with per-partition bias (pad masking; no max subtraction needed)
  stage2 O^T[c,n] = [V|1]^T @ P^T accumulated over l chunks (heads interleaved
  so P^T tiles release early); row 64 = softmax denominator
  normalize via reciprocal + gpsimd partition_broadcast, o_proj to natural
  layout, + bias, DMA out.
Emission is software-pipelined: prep (DMAs + QKV projections) of batch b+1 is
interleaved into the ACT-bound attention phase of batch b.
"""
import sys

sys.path.insert(0, "/opt/trn_rl_repo")
import numpy as np

B, N, L, D, H, C = 32, 512, 1024, 512, 8, 64
NCORES = 8
BLOC = B // NCORES  # 4 batches per core
SCALE = C ** -0.5
MASK_NEG = -87.0
P = 128
NDC = D // P   # 4 d/e chunks
NNC = N // P   # 4 n chunks
LC_SPARSE = 640

_CACHE = {}


def _nspans(l_c):
    # split l_c into moving-operand spans <=512, each >=256 (f32r full rate)
    if l_c == 640:
        return [(0, 384), (384, 640)]
    return [(s, min(s + 512, l_c)) for s in range(0, l_c, 512)]


def _build_nc(l_chunks):
    import concourse.bacc as bacc
    import concourse.tile as tile
    from concourse import mybir

    f32 = mybir.dt.float32
    f32r = mybir.dt.float32r
    EXP = mybir.ActivationFunctionType.Exp
    L_C = l_chunks * P

    nc = bacc.Bacc()
    xqT_d = nc.declare_dram_parameter("xqT", [BLOC, D, N], f32r, isOutput=False)
    xkT_d = nc.declare_dram_parameter("xkT", [BLOC, D, L_C], f32r, isOutput=False)
    rpbT_d = nc.declare_dram_parameter("rpbT", [BLOC, D, L_C], f32, isOutput=False)
    mb_d = nc.declare_dram_parameter("mbias", [BLOC, L_C], f32, isOutput=False)
    Wq = nc.declare_dram_parameter("Wq", [D, D], f32r, isOutput=False)
    Wk = nc.declare_dram_parameter("Wk", [D, D], f32r, isOutput=False)
    Wv = nc.declare_dram_parameter("Wv", [D, D], f32r, isOutput=False)
    Wo = nc.declare_dram_parameter("Wo", [D, D], f32r, isOutput=False)
    bo = nc.declare_dram_parameter("bo", [1, D], f32, isOutput=False)
    out = nc.declare_dram_parameter("out", [BLOC, N, D], f32, isOutput=True)

    with tile.TileContext(nc) as tc:
        with (
            tc.tile_pool(name="consts", bufs=1) as consts,
            tc.tile_pool(name="xin", bufs=2) as xin_pool,
            tc.tile_pool(name="qkt", bufs=2) as qkt_pool,
            tc.tile_pool(name="vp", bufs=2) as vp_pool,
            tc.tile_pool(name="pt", bufs=7) as pt_pool,
            tc.tile_pool(name="ot", bufs=2) as ot_pool,
            tc.tile_pool(name="outst", bufs=3) as outst_pool,
            tc.tile_pool(name="small", bufs=2) as small,
            tc.tile_pool(name="ps_sc", bufs=2, space="PSUM") as ps_sc,
            tc.tile_pool(name="ps_mm", bufs=2, space="PSUM") as ps_mm,
            tc.tile_pool(name="ps_o", bufs=2, space="PSUM") as ps_o,
        ):
            state = {}

            # ---- one-time setup ----
            warm = consts.tile([P, 1], f32, tag="warm")
            nc.vector.memset(warm, 0.0)
            nc.scalar.activation(out=warm, in_=warm, func=EXP, scale=1.0)

            ones8 = consts.tile([P, H], f32, tag="ones8")
            nc.vector.memset(ones8, 1.0)

            wsb = {}

            def load_w(wi, W):
                for k in range(NDC):
                    wt = consts.tile([P, D], f32r, tag=f"w{wi}_{k}", name=f"w{wi}_{k}")
                    nc.sync.dma_start(out=wt, in_=W[k * P:(k + 1) * P, :])
                    wsb[(wi, k)] = wt

            # Wq and batch-0 xqT interleaved by chunk so the first Q-proj
            # matmul group can start after ~0.5 MB of DMA
            xqT0 = []
            for k in range(NDC):
                wt = consts.tile([P, D], f32r, tag=f"w0_{k}", name=f"w0_{k}")
                nc.sync.dma_start(out=wt, in_=Wq[k * P:(k + 1) * P, :])
                wsb[(0, k)] = wt
                t = xin_pool.tile([P, N], f32r, tag=f"xqT{k}", name=f"xqT{k}")
                nc.sync.dma_start(out=t, in_=xqT_d[0, k * P:(k + 1) * P, :])
                xqT0.append(t)
            state[(0, "xqT0")] = xqT0

            bo_row = consts.tile([1, D], f32, tag="bo_row")
            nc.sync.dma_start(out=bo_row, in_=bo[:])
            bo_bc = consts.tile([P, D], f32, tag="bo_bc")
            nc.gpsimd.partition_broadcast(bo_bc, bo_row[0:1, :], channels=P)

            # ---- pipelined prep slices ----
            def prep_slice(b, sl):
                if sl == 0:
                    mb = small.tile([P, l_chunks], f32, tag="mbias")
                    nc.sync.dma_start(
                        out=mb, in_=mb_d[b, :].rearrange("(i p) -> p i", p=P))
                    state[(b, "mbias")] = mb
                    xqT = state.pop((0, "xqT0"), None) if b == 0 else None
                    if xqT is None:
                        xqT = []
                        for k in range(NDC):
                            t = xin_pool.tile([P, N], f32r, tag=f"xqT{k}",
                                              name=f"xqT{k}")
                            nc.sync.dma_start(out=t,
                                              in_=xqT_d[b, k * P:(k + 1) * P, :])
                            xqT.append(t)
                    qT = []
                    for j in range(NDC):
                        pq = ps_mm.tile([P, N], f32, tag="mm", name="pq")
                        for k in range(NDC):
                            nc.tensor.matmul(pq, wsb[(0, k)][:, j * P:(j + 1) * P],
                                             xqT[k], start=(k == 0),
                                             stop=(k == NDC - 1))
                        t = qkt_pool.tile([P, N], f32r, tag=f"qT{j}", name=f"qT{j}")
                        nc.vector.tensor_copy(t, pq)
                        qT.append(t)
                    state[(b, "qT")] = qT
                elif sl == 1:
                    xkT, rpbT = [], []
                    for k in range(NDC):
                        t = xin_pool.tile([P, L_C], f32r, tag=f"xkT{k}",
                                          name=f"xkT{k}")
                        nc.sync.dma_start(out=t, in_=xkT_d[b, k * P:(k + 1) * P, :])
                        xkT.append(t)
                        r = xin_pool.tile([P, L_C], f32, tag=f"rpbT{k}",
                                          name=f"rpbT{k}")
                        nc.sync.dma_start(out=r, in_=rpbT_d[b, k * P:(k + 1) * P, :])
                        rpbT.append(r)
                    state[(b, "xkT")] = xkT
                    state[(b, "rpbT")] = rpbT
                elif sl == 2:
                    xkT = state[(b, "xkT")]
                    rpbT = state.pop((b, "rpbT"))
                    kT = []
                    for j in range(NDC):
                        t = qkt_pool.tile([P, L_C], f32r, tag=f"kT{j}", name=f"kT{j}")
                        for (n0, n1) in _nspans(L_C):
                            pk = ps_mm.tile([P, N], f32, tag="mm", name="pk")
                            for k in range(NDC):
                                nc.tensor.matmul(
                                    pk[:, 0:n1 - n0],
                                    wsb[(1, k)][:, j * P:(j + 1) * P],
                                    xkT[k][:, n0:n1],
                                    start=(k == 0), stop=(k == NDC - 1))
                            nc.vector.tensor_add(
                                t[:, n0:n1], pk[:, 0:n1 - n0], rpbT[j][:, n0:n1])
                        kT.append(t)
                    state[(b, "kT")] = kT
                elif sl == 3:
                    xkT = state.pop((b, "xkT"))
                    vP = []
                    for i in range(l_chunks):
                        pv = ps_mm.tile([P, N], f32, tag="mm", name="pv")
                        for k in range(NDC):
                            nc.tensor.matmul(pv, xkT[k][:, i * P:(i + 1) * P],
                                             wsb[(2, k)], start=(k == 0),
                                             stop=(k == NDC - 1))
                        t = vp_pool.tile([P, H, C + 1], f32r, tag=f"vp{i}",
                                         name=f"vp{i}")
                        nc.vector.tensor_copy(
                            t[:, :, 0:C], pv.rearrange("p (h c) -> p h c", h=H))
                        nc.vector.tensor_copy(t[:, :, C:C + 1], ones8[:, :, None])
                        vP.append(t)
                    state[(b, "vP")] = vP

            def attention_pair(b, j):
                mb = state[(b, "mbias")]
                qT, kT, vP = state[(b, "qT")], state[(b, "kT")], state[(b, "vP")]
                oT = state[(b, "oT")]
                ptiles = []
                for i in range(l_chunks):
                    pss = ps_sc.tile([P, 2 * N], f32, tag="sc", name="pss")
                    for half in range(2):
                        lo = 64 * half
                        nc.tensor.matmul(
                            pss[:, half * N:(half + 1) * N],
                            kT[j][lo:lo + 64, i * P:(i + 1) * P],
                            qT[j][lo:lo + 64, :], start=True, stop=True,
                            tile_position=(lo, 0))
                    pe = pt_pool.tile([P, 2 * N], f32r, tag="pt", name="pe")
                    nc.scalar.activation(out=pe, in_=pss, func=EXP,
                                         bias=mb[:, i:i + 1], scale=SCALE)
                    ptiles.append(pe)
                # stage2, heads interleaved so each ptile releases after 2 reads
                po = {}
                for half in range(2):
                    po[half] = ps_o.tile([C + 1, N], f32, tag="st2", name="po")
                for i in range(l_chunks):
                    for half in range(2):
                        nc.tensor.matmul(po[half], vP[i][:, 2 * j + half, :],
                                         ptiles[i][:, half * N:(half + 1) * N],
                                         start=(i == 0), stop=(i == l_chunks - 1))
                for half in range(2):
                    tr = small.tile([1, N], f32, tag="tr")
                    nc.vector.reciprocal(tr, po[half][C:C + 1, :])
                    trb = small.tile([C, N], f32, tag="trb")
                    nc.gpsimd.partition_broadcast(trb, tr[0:1, :], channels=C)
                    lo = 64 * half
                    nc.vector.tensor_mul(oT[j][lo:lo + 64, :], po[half][0:C, :], trb)

            def oproj(b):
                oT = state.pop((b, "oT"))
                for key in ("mbias", "qT", "kT", "vP"):
                    state.pop((b, key), None)
                for m in range(NNC):
                    pf = ps_mm.tile([P, N], f32, tag="mm", name="pf")
                    for j in range(NDC):
                        nc.tensor.matmul(pf, oT[j][:, m * P:(m + 1) * P],
                                         wsb[(3, j)], start=(j == 0),
                                         stop=(j == NDC - 1))
                    to = outst_pool.tile([P, D], f32, tag="outst", name="to")
                    nc.vector.tensor_add(to, pf, bo_bc)
                    nc.sync.dma_start(out=out[b, m * P:(m + 1) * P, :], in_=to)

            # ---- main pipeline ----
            prep_slice(0, 0)
            load_w(1, Wk)
            prep_slice(0, 1)
            load_w(2, Wv)
            load_w(3, Wo)
            prep_slice(0, 2)
            prep_slice(0, 3)
            for b in range(BLOC):
                state[(b, "oT")] = [
                    ot_pool.tile([P, N], f32r, tag=f"oT{j}", name=f"oT{j}")
                    for j in range(NDC)]
                for j in range(NDC):
                    attention_pair(b, j)
                    if b + 1 < BLOC:
                        prep_slice(b + 1, j)
                oproj(b)

    nc.compile()
    return nc


def _get_nc(l_chunks=LC_SPARSE // P):
    key = ("nc", l_chunks)
    if key not in _CACHE:
        _CACHE[key] = _build_nc(l_chunks)
    return _CACHE[key]


def kernel(x_q, x_kv, pad_mask, Wq, Wk, Wv, Wo, bo, rpb):
    from concourse.bass_utils import run_bass_kernel_spmd

    x_q = np.asarray(x_q, dtype=np.float32)
    x_kv = np.asarray(x_kv, dtype=np.float32)
    pad_mask = np.asarray(pad_mask).astype(bool)
    rpb2 = np.asarray(rpb, np.float32).reshape(L, D)

    counts = (~pad_mask).sum(axis=1)
    L_C = LC_SPARSE if counts.max() <= LC_SPARSE else L
    nc = _get_nc(L_C // P)

    shared = {
        "Wq": np.asarray(Wq, np.float32), "Wk": np.asarray(Wk, np.float32),
        "Wv": np.asarray(Wv, np.float32), "Wo": np.asarray(Wo, np.float32),
        "bo": np.asarray(bo, np.float32).reshape(1, D),
    }
    in_maps = []
    for c in range(NCORES):
        sl = slice(c * BLOC, (c + 1) * BLOC)
        xkT = np.zeros((BLOC, D, L_C), np.float32)
        rpbT = np.zeros((BLOC, D, L_C), np.float32)
        mb = np.full((BLOC, L_C), MASK_NEG, np.float32)
        for b in range(BLOC):
            g = c * BLOC + b
            idx = np.nonzero(~pad_mask[g])[0]
            cnt = len(idx)
            xkT[b, :, :cnt] = x_kv[g, idx, :].T
            rpbT[b, :, :cnt] = rpb2[idx, :].T
            mb[b, :cnt] = 0.0
        in_maps.append({
            "xqT": np.ascontiguousarray(x_q[sl].transpose(0, 2, 1)),
            "xkT": xkT, "rpbT": rpbT, "mbias": mb,
            **shared,
        })
    res = run_bass_kernel_spmd(nc, in_maps, list(range(NCORES)))
    return np.concatenate([res.results[c]["out"] for c in range(NCORES)], axis=0)


# revision 13
# speedup vs baseline: 1.0324x; 1.0324x over previous
"""MultiHeadAttention (cross-attention, B=32 N=512 L=1024 D=512 H=8) on 8 TRN2 cores.

Strategy: data parallelism (4 batches/core) + host-side sparsity compaction.

Host prep (inside kernel(), plain numpy):
  - per batch, gather the unmasked K/V positions (~50% of L=1024), pad to
    L_C=640 (5*128); padded slots get zero K/V rows and a -87 exp bias so they
    vanish from the softmax exactly like reference's -inf masking
  - rpb rows gathered the same way; x_q / x_kv / rpb pre-TRANSPOSED on host so
    the device needs no PE transposes at all
Device per-core dataflow (all matmuls float32r, 1 cycle/row on PE):
  Q^T/K^T (+rpb^T via DVE add) head-major; V natural with interleaved ones col
  scores S^T[l,n] per head-pair packed via tile_position (K=64 row groups),
  both heads' scores in one [128,1024] PSUM tile -> single exp per (pair,chunk)
  exp on ACT with per-partition bias (pad masking; no max subtraction needed)
  stage2 O^T[c,n] = [V|1]^T @ P^T accumulated over l chunks (heads interleaved
  so P^T tiles release early); row 64 = softmax denominator
  normalize via reciprocal + gpsimd partition_broadcast, o_proj to natural
  layout, + bias, DMA out.
Emission is software-pipelined: prep (DMAs + QKV projections) of batch b+1 is
interleaved into the ACT-bound attention phase of batch b.
"""
import sys

sys.path.insert(0, "/opt/trn_rl_repo")
import numpy as np

B, N, L, D, H, C = 32, 512, 1024, 512, 8, 64
NCORES = 8
BLOC = B // NCORES  # 4 batches per core
SCALE = C ** -0.5
MASK_NEG = -87.0
P = 128
NDC = D // P   # 4 d/e chunks
NNC = N // P   # 4 n chunks
LC_SPARSE = 640

_CACHE = {}


def _nspans(l_c):
    # split l_c into moving-operand spans <=512, each >=256 (f32r full rate)
    if l_c == 640:
        return [(0, 384), (384, 640)]
    return [(s, min(s + 512, l_c)) for s in range(0, l_c, 512)]


def _build_nc(l_chunks):
    import concourse.bacc as bacc
    import concourse.tile as tile
    from concourse import mybir

    f32 = mybir.dt.float32
    f32r = mybir.dt.float32r
    EXP = mybir.ActivationFunctionType.Exp
    L_C = l_chunks * P

    nc = bacc.Bacc()
    xqT_d = nc.declare_dram_parameter("xqT", [BLOC, D, N], f32r, isOutput=False)
    xkT_d = nc.declare_dram_parameter("xkT", [BLOC, D, L_C], f32r, isOutput=False)
    rpbT_d = nc.declare_dram_parameter("rpbT", [BLOC, D, L_C], f32, isOutput=False)
    mb_d = nc.declare_dram_parameter("mbias", [BLOC, L_C], f32, isOutput=False)
    Wq = nc.declare_dram_parameter("Wq", [D, D], f32r, isOutput=False)
    Wk = nc.declare_dram_parameter("Wk", [D, D], f32r, isOutput=False)
    Wv = nc.declare_dram_parameter("Wv", [D, D], f32r, isOutput=False)
    Wo = nc.declare_dram_parameter("Wo", [D, D], f32r, isOutput=False)
    bo = nc.declare_dram_parameter("bo", [1, D], f32, isOutput=False)
    out = nc.declare_dram_parameter("out", [BLOC, N, D], f32, isOutput=True)

    with tile.TileContext(nc) as tc:
        with (
            tc.tile_pool(name="consts", bufs=1) as consts,
            tc.tile_pool(name="xin", bufs=2) as xin_pool,
            tc.tile_pool(name="qkt", bufs=2) as qkt_pool,
            tc.tile_pool(name="vp", bufs=2) as vp_pool,
            tc.tile_pool(name="pt", bufs=7) as pt_pool,
            tc.tile_pool(name="ot", bufs=2) as ot_pool,
            tc.tile_pool(name="outst", bufs=3) as outst_pool,
            tc.tile_pool(name="small", bufs=2) as small,
            tc.tile_pool(name="ps_sc", bufs=2, space="PSUM") as ps_sc,
            tc.tile_pool(name="ps_mm", bufs=2, space="PSUM") as ps_mm,
            tc.tile_pool(name="ps_o", bufs=2, space="PSUM") as ps_o,
        ):
            state = {}

            # ---- one-time setup ----
            warm = consts.tile([P, 1], f32, tag="warm")
            nc.vector.memset(warm, 0.0)
            nc.scalar.activation(out=warm, in_=warm, func=EXP, scale=1.0)

            ones8 = consts.tile([P, H], f32, tag="ones8")
            nc.vector.memset(ones8, 1.0)

            wsb = {}

            def load_w(wi, W):
                for k in range(NDC):
                    wt = consts.tile([P, D], f32r, tag=f"w{wi}_{k}", name=f"w{wi}_{k}")
                    nc.sync.dma_start(out=wt, in_=W[k * P:(k + 1) * P, :])
                    wsb[(wi, k)] = wt

            # Wq and batch-0 xqT interleaved by chunk so the first Q-proj
            # matmul group can start after ~0.5 MB of DMA
            xqT0 = []
            for k in range(NDC):
                wt = consts.tile([P, D], f32r, tag=f"w0_{k}", name=f"w0_{k}")
                nc.sync.dma_start(out=wt, in_=Wq[k * P:(k + 1) * P, :])
                wsb[(0, k)] = wt
                t = xin_pool.tile([P, N], f32r, tag=f"xqT{k}", name=f"xqT{k}")
                nc.sync.dma_start(out=t, in_=xqT_d[0, k * P:(k + 1) * P, :])
                xqT0.append(t)
            state[(0, "xqT0")] = xqT0

            bo_row = consts.tile([1, D], f32, tag="bo_row")
            nc.sync.dma_start(out=bo_row, in_=bo[:])
            bo_bc = consts.tile([P, D], f32, tag="bo_bc")
            nc.gpsimd.partition_broadcast(bo_bc, bo_row[0:1, :], channels=P)

            # ---- pipelined prep slices ----
            def prep_slice(b, sl):
                if sl == 0:
                    mb = small.tile([P, l_chunks], f32, tag="mbias")
                    nc.sync.dma_start(
                        out=mb, in_=mb_d[b, :].rearrange("(i p) -> p i", p=P))
                    state[(b, "mbias")] = mb
                    xqT = state.pop((0, "xqT0"), None) if b == 0 else None
                    if xqT is None:
                        xqT = []
                        for k in range(NDC):
                            t = xin_pool.tile([P, N], f32r, tag=f"xqT{k}",
                                              name=f"xqT{k}")
                            nc.sync.dma_start(out=t,
                                              in_=xqT_d[b, k * P:(k + 1) * P, :])
                            xqT.append(t)
                    qT = []
                    for j in range(NDC):
                        pq = ps_mm.tile([P, N], f32, tag="mm", name="pq")
                        for k in range(NDC):
                            nc.tensor.matmul(pq, wsb[(0, k)][:, j * P:(j + 1) * P],
                                             xqT[k], start=(k == 0),
                                             stop=(k == NDC - 1))
                        t = qkt_pool.tile([P, N], f32r, tag=f"qT{j}", name=f"qT{j}")
                        nc.scalar.copy(t, pq)
                        qT.append(t)
                    state[(b, "qT")] = qT
                elif sl == 1:
                    xkT, rpbT = [], []
                    for k in range(NDC):
                        t = xin_pool.tile([P, L_C], f32r, tag=f"xkT{k}",
                                          name=f"xkT{k}")
                        nc.sync.dma_start(out=t, in_=xkT_d[b, k * P:(k + 1) * P, :])
                        xkT.append(t)
                        r = xin_pool.tile([P, L_C], f32, tag=f"rpbT{k}",
                                          name=f"rpbT{k}")
                        nc.sync.dma_start(out=r, in_=rpbT_d[b, k * P:(k + 1) * P, :])
                        rpbT.append(r)
                    state[(b, "xkT")] = xkT
                    state[(b, "rpbT")] = rpbT
                elif sl == 2:
                    xkT = state[(b, "xkT")]
                    rpbT = state.pop((b, "rpbT"))
                    kT = []
                    for j in range(NDC):
                        t = qkt_pool.tile([P, L_C], f32r, tag=f"kT{j}", name=f"kT{j}")
                        for (n0, n1) in _nspans(L_C):
                            pk = ps_mm.tile([P, N], f32, tag="mm", name="pk")
                            for k in range(NDC):
                                nc.tensor.matmul(
                                    pk[:, 0:n1 - n0],
                                    wsb[(1, k)][:, j * P:(j + 1) * P],
                                    xkT[k][:, n0:n1],
                                    start=(k == 0), stop=(k == NDC - 1))
                            nc.vector.tensor_add(
                                t[:, n0:n1], pk[:, 0:n1 - n0], rpbT[j][:, n0:n1])
                        kT.append(t)
                    state[(b, "kT")] = kT
                elif sl == 3:
                    xkT = state.pop((b, "xkT"))
                    vP = []
                    for i in range(l_chunks):
                        pv = ps_mm.tile([P, N], f32, tag="mm", name="pv")
                        for k in range(NDC):
                            nc.tensor.matmul(pv, xkT[k][:, i * P:(i + 1) * P],
                                             wsb[(2, k)], start=(k == 0),
                                             stop=(k == NDC - 1))
                        t = vp_pool.tile([P, H, C + 1], f32r, tag=f"vp{i}",
                                         name=f"vp{i}")
                        nc.vector.tensor_copy(
                            t[:, :, 0:C], pv.rearrange("p (h c) -> p h c", h=H))
                        nc.vector.tensor_copy(t[:, :, C:C + 1], ones8[:, :, None])
                        vP.append(t)
                    state[(b, "vP")] = vP

            def attention_pair(b, j):
                mb = state[(b, "mbias")]
                qT, kT, vP = state[(b, "qT")], state[(b, "kT")], state[(b, "vP")]
                oT = state[(b, "oT")]
                ptiles = []
                for i in range(l_chunks):
                    pss = ps_sc.tile([P, 2 * N], f32, tag="sc", name="pss")
                    for half in range(2):
                        lo = 64 * half
                        nc.tensor.matmul(
                            pss[:, half * N:(half + 1) * N],
                            kT[j][lo:lo + 64, i * P:(i + 1) * P],
                            qT[j][lo:lo + 64, :], start=True, stop=True,
                            tile_position=(lo, 0))
                    pe = pt_pool.tile([P, 2 * N], f32r, tag="pt", name="pe")
                    nc.scalar.activation(out=pe, in_=pss, func=EXP,
                                         bias=mb[:, i:i + 1], scale=SCALE)
                    ptiles.append(pe)
                # stage2, heads interleaved so each ptile releases after 2 reads
                po = {}
                for half in range(2):
                    po[half] = ps_o.tile([C + 1, N], f32, tag="st2", name="po")
                for i in range(l_chunks):
                    for half in range(2):
                        nc.tensor.matmul(po[half], vP[i][:, 2 * j + half, :],
                                         ptiles[i][:, half * N:(half + 1) * N],
                                         start=(i == 0), stop=(i == l_chunks - 1))
                for half in range(2):
                    tr = small.tile([1, N], f32, tag="tr")
                    nc.vector.reciprocal(tr, po[half][C:C + 1, :])
                    trb = small.tile([C, N], f32, tag="trb")
                    nc.gpsimd.partition_broadcast(trb, tr[0:1, :], channels=C)
                    lo = 64 * half
                    nc.vector.tensor_mul(oT[j][lo:lo + 64, :], po[half][0:C, :], trb)

            def oproj(b):
                oT = state.pop((b, "oT"))
                for key in ("mbias", "qT", "kT", "vP"):
                    state.pop((b, key), None)
                for m in range(NNC):
                    pf = ps_mm.tile([P, N], f32, tag="mm", name="pf")
                    for j in range(NDC):
                        nc.tensor.matmul(pf, oT[j][:, m * P:(m + 1) * P],
                                         wsb[(3, j)], start=(j == 0),
                                         stop=(j == NDC - 1))
                    to = outst_pool.tile([P, D], f32, tag="outst", name="to")
                    nc.vector.tensor_add(to, pf, bo_bc)
                    nc.sync.dma_start(out=out[b, m * P:(m + 1) * P, :], in_=to)

            # ---- main pipeline ----
            prep_slice(0, 0)
            load_w(1, Wk)
            prep_slice(0, 1)
            load_w(2, Wv)
            load_w(3, Wo)
            prep_slice(0, 2)
            prep_slice(0, 3)
            for b in range(BLOC):
                state[(b, "oT")] = [
                    ot_pool.tile([P, N], f32r, tag=f"oT{j}", name=f"oT{j}")
                    for j in range(NDC)]
                for j in range(NDC):
                    attention_pair(b, j)
                    if b + 1 < BLOC:
                        prep_slice(b + 1, j)
                oproj(b)

    nc.compile()
    return nc


def _get_nc(l_chunks=LC_SPARSE // P):
    key = ("nc", l_chunks)
    if key not in _CACHE:
        _CACHE[key] = _build_nc(l_chunks)
    return _CACHE[key]


def kernel(x_q, x_kv, pad_mask, Wq, Wk, Wv, Wo, bo, rpb):
    from concourse.bass_utils import run_bass_kernel_spmd

    x_q = np.asarray(x_q, dtype=np.float32)
    x_kv = np.asarray(x_kv, dtype=np.float32)
    pad_mask = np.asarray(pad_mask).astype(bool)
    rpb2 = np.asarray(rpb, np.float32).reshape(L, D)

    counts = (~pad_mask).sum(axis=1)
    L_C = LC_SPARSE if counts.max() <= LC_SPARSE else L
    nc = _get_nc(L_C // P)

    shared = {
        "Wq": np.asarray(Wq, np.float32), "Wk": np.asarray(Wk, np.float32),
        "Wv": np.asarray(Wv, np.float32), "Wo": np.asarray(Wo, np.float32),
        "bo": np.asarray(bo, np.float32).reshape(1, D),
    }
    in_maps = []
    for c in range(NCORES):
        sl = slice(c * BLOC, (c + 1) * BLOC)
        xkT = np.zeros((BLOC, D, L_C), np.float32)
        rpbT = np.zeros((BLOC, D, L_C), np.float32)
        mb = np.full((BLOC, L_C), MASK_NEG, np.float32)
        for b in range(BLOC):
            g = c * BLOC + b
            idx = np.nonzero(~pad_mask[g])[0]
            cnt = len(idx)
            xkT[b, :, :cnt] = x_kv[g, idx, :].T
            rpbT[b, :, :cnt] = rpb2[idx, :].T
            mb[b, :cnt] = 0.0
        in_maps.append({
            "xqT": np.ascontiguousarray(x_q[sl].transpose(0, 2, 1)),
            "xkT": xkT, "rpbT": rpbT, "mbias": mb,
            **shared,
        })
    res = run_bass_kernel_spmd(nc, in_maps, list(range(NCORES)))
    return np.concatenate([res.results[c]["out"] for c in range(NCORES)], axis=0)
